# revision 20
# baseline (speedup 1.0000x reference)
"""Dense bilateral energy loss (DenseEnergyLoss) on 8 Trainium2 cores.

Math (per image n, after 2x downsample => oh=ow=64, P=4096):
  feat[p] = (x/40, y/40, r/15, g/15, b/15)          # 5 dims
  A[p,q]  = exp(-(||feat_p - feat_q||^2)/2)          # dense [P,P]
  AS[k,q] = sum_p seg_m[k,p] * A[p,q]                # A symmetric
  loss    = -0.05 * sum_{k,q} seg_m[k,q]*gate[q]*AS[k,q] / (N*P)

Device work per core (half an image: 2048 of the 4096 q columns):
  MM1 (PE):  dot[p,q] = -0.5*d2[p,q] via bf16 hi/lo-split contraction.
             Contraction zero-padded 21->128 rows: same cycles (cost is the
             512-col stream) but keeps the PE HAM activity monitor busy so
             the clock stays at 2.4GHz (thin matmuls let it fall to 1.2).
  EXP (ACT): A = exp(dot), bf16, 1024-wide to amortize fixed access costs.
  MM2 (PE):  AS^T accumulation, col-tiled 4x: tile_position=(0,32j) runs 4
             p-blocks concurrently (out[21,512] slices at PSUM partitions
             32j), ~4x the A-consumption rate of plain [21,512] matmuls.
Host (numpy): resizes (2x2 avgpool / [::2,::2]), gate, seg_m, features,
hi/lo split, final masked reduction of AS. All cheap elementwise work.
"""

import sys

sys.path.insert(0, "/opt/trn_rl_repo")

import numpy as np
import ml_dtypes

# ---------------- problem constants (hardcoded per contract) ---------------
N, K, H, W = 4, 21, 128, 128
OH, OW = 64, 64
P = OH * OW  # 4096
WEIGHT = 0.1
SIGMA_RGB = 15.0
SIGMA_XY = 80.0
SCALE = 0.5
IGNORE_LABEL = 255
N_CORES = 8
QCOLS = P // 2  # q columns per core (2 cores per image)
QB = 512  # q tile width (one PSUM bank)
NQ = QCOLS // QB  # 4 q-bands per core
NPB = P // 128  # 32 p-blocks
CROWS = 128  # MM1 contraction rows (21 real + zero padding)

BF16 = ml_dtypes.bfloat16

_PROGRAM = None  # built once per process


def _hilo(x):
    """Split fp32 array into bf16 hi + bf16 lo with x ~= hi + lo."""
    x = np.asarray(x, np.float32)
    hi = x.astype(BF16)
    lo = (x - hi.astype(np.float32)).astype(BF16)
    return hi, lo


def _patch_tile_drain():
    """This container's walrus allows only one sync wait per CTRL (Drain/Nop)
    instruction; Tile's exit drain attaches one wait per DMA-HW queue sem.
    Split the extra waits onto dedicated nops."""
    from concourse import mybir
    from concourse.tile import TileContext
    from concourse.vector_clock import ScopedClock

    if getattr(TileContext, "_drain_split_patched", False):
        return

    def _drain_and_barrier(self, tick_clock, wait_clock):
        nc = self.nc
        drain_inst = nc.sync.drain()
        wait_clock.add_sem_waits(
            drain_inst.ins, ScopedClock({None: tick_clock.global_clock})
        )
        si = drain_inst.ins.sync_info
        waits = list(si.on_wait) if si is not None else []
        if len(waits) > 1:
            del si.on_wait[1:]
            for w in waits[1:]:
                n = nc.sync.nop(nofuse=True, hint="drain_split")
                n.ins.sync_info = mybir.SyncInfo(on_wait=[w], on_update=[])
        nc.all_engine_barrier()
        popped = nc._tile_sem_poison_stack.pop()
        assert popped is self._sem_poison
        nc.clear_and_free_semaphores(list(self.sems.allocated().values()))
        nc.all_engine_barrier()

    TileContext._drain_and_barrier = _drain_and_barrier
    TileContext._drain_split_patched = True


def _split_multi_waits(nc):
    """This walrus build supports one sync-wait per instruction. Hoist extra
    waits onto dedicated same-engine nops placed right before the owner."""
    from concourse import mybir

    ctr = 0
    for fn in nc.m.functions:
        for blk in fn.blocks:
            insts = blk.instructions
            new = []
            changed = False
            for inst in insts:
                si = inst.sync_info
                if si is not None and si.on_wait is not None and len(si.on_wait) > 1:
                    waits = list(si.on_wait)
                    for w in waits[:-1]:
                        ctr += 1
                        new.append(
                            mybir.InstNoOp(
                                name=f"WSPLIT-{ctr}",
                                engine=inst.engine,
                                ins=[],
                                outs=[],
                                sync_info=mybir.SyncInfo(
                                    on_wait=[w], on_update=[]
                                ),
                                text_hint="wait_split",
                                bass_nofuse=True,
                            )
                        )
                    si.on_wait = [waits[-1]]
                    inst.sync_info = si
                    changed = True
                new.append(inst)
            if changed:
                blk.instructions = new


def _build_program():
    global _PROGRAM
    if _PROGRAM is not None:
        return _PROGRAM

    _patch_tile_drain()
    import concourse.bass as bass
    from concourse import mybir
    from concourse.tile import TileContext

    nc = bass.Bass("TRN2")
    f32 = mybir.dt.float32
    bf16 = mybir.dt.bfloat16

    # MM1 stationary source: real 21 contraction rows (padded on device)
    flt = nc.dram_tensor("flt", [32, P], bf16, kind="ExternalInput")
    # MM1 moving source: rows 0-20 real, 21-63 zero (device copies 64->128)
    frt = nc.dram_tensor("frt", [64, QCOLS], bf16, kind="ExternalInput")
    # seg_m^T pre-arranged [128, NPB*21]: st[p, pb*21+k] = seg_m[k, pb*128+p]
    st = nc.dram_tensor("st", [128, NPB * 21], bf16, kind="ExternalInput")
    # output: AS[k, q] for this core's q columns (bands 0..NQ-2)
    out = nc.dram_tensor("out", [21, QCOLS], f32, kind="ExternalOutput")
    # last band leaves the 4 col-group partials unsummed (host adds them):
    # evacuated via one ACT copy so the kernel tail skips the DVE chain
    out3 = nc.dram_tensor("out3", [117, QB], f32, kind="ExternalOutput")

    with TileContext(nc) as tc:
        with (
            tc.tile_pool(name="const", bufs=1) as const,
            tc.tile_pool(name="apool", bufs=6) as apool,
            tc.tile_pool(name="osb", bufs=2) as osb,
            tc.tile_pool(name="dotps", bufs=3, space="PSUM") as dotps,
            tc.tile_pool(name="outps", bufs=2, space="PSUM") as outps,
        ):
            flt_s = const.tile([CROWS, P], bf16)
            frt_s = const.tile([CROWS, QCOLS], bf16)
            st_s = const.tile([128, NPB * 21], bf16)
            # Pad rows built on device from the DMA'd rows via DVE bf16
            # copies (4x mode) instead of shipping 128-row tensors: frt pads
            # must be exact zero (rows 21-63 are zero, copied to 64-127);
            # flt pads only need to be finite (multiplied by frt zeros).
            nc.sync.dma_start(out=flt_s[0:32, 0 : P // 2], in_=flt[:, 0 : P // 2])
            nc.sync.dma_start(out=flt_s[0:32, P // 2 : P], in_=flt[:, P // 2 : P])
            nc.sync.dma_start(out=frt_s[0:32, :], in_=frt[0:32, :])
            nc.sync.dma_start(out=frt_s[32:64, :], in_=frt[32:64, :])
            nc.sync.dma_start(out=st_s, in_=st[:, :])
            # frt pad rows must be EXACT zero: rows 32:64 arrive zero from
            # host, replicate that zero block upward (flt pads merely need
            # to be finite, so copying real rows there is fine).
            nc.vector.tensor_copy(frt_s[64:96, :], frt_s[32:64, :])
            nc.vector.tensor_copy(frt_s[96:128, :], frt_s[32:64, :])

            def emit_mm2(out_ps, pg, a_pair):
                for j in range(4):  # col-tiled MM2, 4 p-blocks at once
                    pb = pg * 4 + j
                    nc.tensor.matmul(
                        out_ps[32 * j : 32 * j + 21, :],
                        lhsT=st_s[:, pb * 21 : (pb + 1) * 21],
                        rhs=a_pair[j // 2][:, (j % 2) * QB : (j % 2 + 1) * QB],
                        tile_position=(0, 32 * j),
                        start=(pg == 0),
                        stop=(pg == NPB // 4 - 1),
                    )

            for qb in range(NQ):
                out_ps = outps.tile([128, QB], f32)
                pending = None  # software pipeline: MM2s trail by one group
                for pg in range(NPB // 4):  # groups of 4 p-blocks
                    if qb == 0:  # chunk-wise flt pad fill, ahead of first use
                        c0, c1 = pg * 512, (pg + 1) * 512
                        for b in range(32, CROWS, 32):
                            nc.vector.tensor_copy(
                                flt_s[b : b + 32, c0:c1], flt_s[0:32, c0:c1]
                            )
                    a_t = []
                    for half in range(2):  # 2 dot pairs of [128, 1024]
                        dot_ps = dotps.tile([128, 2 * QB], f32)
                        for j in range(2):
                            pb = pg * 4 + half * 2 + j
                            nc.tensor.matmul(
                                dot_ps[:, j * QB : (j + 1) * QB],
                                lhsT=flt_s[:, pb * 128 : (pb + 1) * 128],
                                rhs=frt_s[:, qb * QB : (qb + 1) * QB],
                                start=True,
                                stop=True,
                            )
                        at = apool.tile([128, 2 * QB], bf16)
                        nc.scalar.activation(
                            at, dot_ps, mybir.ActivationFunctionType.Exp
                        )
                        a_t.append(at)
                    if pending is not None:
                        emit_mm2(out_ps, pending[0], pending[1])
                    pending = (pg, a_t)
                emit_mm2(out_ps, pending[0], pending[1])
                if qb < NQ - 1:
                    # sum the 4 col-group partials [21,512] -> out band
                    # (DVE may read at most one PSUM operand per instruction)
                    t0 = osb.tile([21, QB], f32, tag="t0")
                    nc.vector.tensor_copy(t0, out_ps[0:21, :])
                    t1 = osb.tile([21, QB], f32, tag="t1")
                    nc.vector.tensor_add(t1, t0, out_ps[32:53, :])
                    t2 = osb.tile([21, QB], f32, tag="t2")
                    nc.vector.tensor_add(t2, t1, out_ps[64:85, :])
                    out_sb = osb.tile([21, QB], f32, tag="osum")
                    nc.vector.tensor_add(out_sb, t2, out_ps[96:117, :])
                    nc.sync.dma_start(
                        out=out[:, qb * QB : (qb + 1) * QB], in_=out_sb
                    )
                else:
                    # last band: single ACT evacuation (ACT is idle by now),
                    # partials summed on host — shortens the kernel tail
                    o3 = osb.tile([117, QB], f32, tag="o3")
                    nc.scalar.copy(o3, out_ps[0:117, :])
                    nc.sync.dma_start(out=out3[:, :], in_=o3)

    _split_multi_waits(nc)
    _PROGRAM = nc
    return nc


def _host_prep(images, segmentations, ROIs, seg_label):
    """Resizes, gate, seg_m, bilateral features + hi/lo split. All fp32."""
    images = np.asarray(images, np.float32)
    segmentations = np.asarray(segmentations, np.float32)
    ROIs = np.asarray(ROIs, np.float32)
    seg_label = np.asarray(seg_label, np.float32)

    # nearest resize (scale 0.5, floor(dst*2)) == [::2, ::2]
    img_s = images[:, :, ::2, ::2]  # [N,3,64,64]
    roi_s = ROIs[:, ::2, ::2]  # [N,64,64]
    lab_s = seg_label[:, 0, ::2, ::2]  # [N,64,64]
    # bilinear (align_corners=False, scale 0.5) == 2x2 average pooling
    s = segmentations.reshape(N, K, OH, 2, OW, 2)
    seg_s = 0.25 * (s[:, :, :, 0, :, 0] + s[:, :, :, 0, :, 1]
                    + s[:, :, :, 1, :, 0] + s[:, :, :, 1, :, 1])

    unlabel = lab_s.astype(np.int32) == IGNORE_LABEL
    gate = roi_s - seg_s.max(axis=1)
    gate = np.where(unlabel, np.float32(1.0), gate)
    gate = np.maximum(gate, 0.0).reshape(N, P)  # [N,P]

    seg_m = (seg_s * roi_s[:, None]).reshape(N, K, P)  # [N,K,P]

    sxy = SIGMA_XY * SCALE
    ys, xs = np.meshgrid(np.arange(OH, dtype=np.float32),
                         np.arange(OW, dtype=np.float32), indexing="ij")
    xy = np.stack([xs.ravel(), ys.ravel()], axis=1) / sxy  # [P,2]
    rgb = img_s.reshape(N, 3, P).transpose(0, 2, 1) / SIGMA_RGB  # [N,P,3]
    feat = np.concatenate(
        [np.broadcast_to(xy, (N, P, 2)), rgb], axis=-1
    ).astype(np.float32)  # [N,P,5]

    sq = np.sum(feat * feat, axis=-1)  # [N,P]
    ones = np.ones((N, P, 1), np.float32)
    mhalf = (-0.5 * sq)[:, :, None]
    featL = np.concatenate([feat, ones, mhalf], axis=-1)  # [N,P,7]
    featR = np.concatenate([feat, mhalf, ones], axis=-1)  # [N,P,7]

    hiL, loL = _hilo(featL)
    hiR, loR = _hilo(featR)
    # 21 real contraction rows: dot = hiL.hiR + hiL.loR + loL.hiR.
    # (The device pads these to 128 rows with zeros so the PE activity
    # monitor sees dense matmuls and keeps the clock at 2.4GHz.)
    fLT = np.zeros((N, 32, P), BF16)
    fRT = np.zeros((N, 64, P), BF16)
    fLT[:, 0:21] = np.concatenate([hiL, hiL, loL], axis=-1).transpose(0, 2, 1)
    fRT[:, 0:21] = np.concatenate([hiR, loR, hiR], axis=-1).transpose(0, 2, 1)

    # st arrangement [N, 128, NPB*21]
    st = (
        seg_m.astype(BF16)
        .transpose(0, 2, 1)  # [N,P,K]
        .reshape(N, NPB, 128, K)
        .transpose(0, 2, 1, 3)  # [N,128,NPB,K]
        .reshape(N, 128, NPB * K)
        .copy()
    )
    return seg_m, gate, fLT, fRT, st


def kernel(images, segmentations, ROIs, seg_label):
    from concourse.bass_utils import run_bass_kernel_spmd

    seg_m, gate, fLT, fRT, st = _host_prep(
        images, segmentations, ROIs, seg_label
    )

    nc = _build_program()
    in_maps = []
    for c in range(N_CORES):
        n, half = c // 2, c % 2
        qs = slice(half * QCOLS, (half + 1) * QCOLS)
        in_maps.append(
            {
                "flt": np.ascontiguousarray(fLT[n]),
                "frt": np.ascontiguousarray(fRT[n][:, qs]),
                "st": st[n],
            }
        )

    res = run_bass_kernel_spmd(nc, in_maps, core_ids=list(range(N_CORES)))

    AS = np.empty((N, K, P), np.float64)
    for c in range(N_CORES):
        n, half = c // 2, c % 2
        o = res.results[c]["out"].astype(np.float64)
        o3 = res.results[c]["out3"].astype(np.float64)
        # last band: sum the 4 col-group partial rows the device left split
        o[:, (NQ - 1) * QB :] = (
            o3[0:21] + o3[32:53] + o3[64:85] + o3[96:117]
        )
        AS[n, :, half * QCOLS : (half + 1) * QCOLS] = o

    total = np.sum(seg_m.astype(np.float64) * gate[:, None].astype(np.float64) * AS)
    loss = WEIGHT * (-0.5) * total / (N * P)
    return np.array(loss, dtype=np.float32)
